# revision 1
# baseline (speedup 1.0000x reference)
"""LSTM encoder kernel for Trainium2 (8 NeuronCores, data-parallel over batch).

Host-side prep folds the embedding lookup + input projection into a single
120-row table XC (30 letter tokens x 4 state tokens), gate-reordered so each
H-quarter's PSUM bank pair holds [g|i|f|o]. Per step the device does:
  gates_q = sum_k hT[k].T @ W[k]   (PE, PSUM accumulate; h is the stationary)
  gates_q += xp_q                  (DVE; xp rows gathered from XC by index)
  c, h elementwise on ACT/DVE; h fed back via blocked bf16 transpose-DMA.
"""

from contextlib import ExitStack

import ml_dtypes
import numpy as np

import concourse.bacc as bacc
import concourse.bass as bass
import concourse.mybir as mybir
import concourse.tile as tile
from concourse.bass_utils import run_bass_kernel_spmd

F32 = mybir.dt.float32
BF16 = mybir.dt.bfloat16
I32 = mybir.dt.int32

B, S, E, H = 256, 256, 256, 1024
NCORES = 8
BL = B // NCORES          # 32 batch rows per core
NK = H // 128             # 8 contraction tiles
NQ = 4                    # H quarters
QH = H // NQ              # 256 h-units per quarter
GATE_BLOCKS = [0, 1, 2, 3]  # our order [i, f, g, o] == pytorch row blocks

XP_MODE = "dve"           # "dve": indirect-DMA gather + DVE add; "pe": matmul inject

_cache = {}


def _build(steps: int, xp_mode: str = XP_MODE):
    nc = bacc.Bacc("TRN2", target_bir_lowering=False, debug=False, enable_asserts=True)

    w_dram = nc.dram_tensor("W", [H, 4 * H], BF16, kind="ExternalInput")
    xc_dram = nc.dram_tensor("XC", [120, 4 * H], BF16, kind="ExternalInput")
    if xp_mode == "dve":
        idx_dram = nc.dram_tensor("IDX", [BL, steps], I32, kind="ExternalInput")
    else:
        oh_dram = nc.dram_tensor("OH", [120, BL * steps], BF16, kind="ExternalInput")
    hid_dram = nc.dram_tensor("hid", [BL, steps, H], F32, kind="ExternalOutput")
    cell_dram = nc.dram_tensor("cell", [BL, steps, H], F32, kind="ExternalOutput")

    Tanh = mybir.ActivationFunctionType.Tanh
    Sigmoid = mybir.ActivationFunctionType.Sigmoid

    with tile.TileContext(nc) as tc, ExitStack() as ctx:
        resident = ctx.enter_context(tc.tile_pool(name="resident", bufs=1))
        psum_pool = ctx.enter_context(tc.tile_pool(name="psum", bufs=1, space="PSUM"))
        act_pool = ctx.enter_context(tc.tile_pool(name="act", bufs=4))
        h_pool = ctx.enter_context(tc.tile_pool(name="h", bufs=4))
        xp_pool = ctx.enter_context(tc.tile_pool(name="xp", bufs=4))

        w_sb = resident.tile([128, NK, 4 * H], BF16)
        w_view = w_dram.ap().rearrange("(k p) n -> k p n", p=128)
        for k in range(NK):
            # alternate HWDGE rings (SP / ACT) to halve the prologue wall time
            eng = nc.sync if k % 2 == 0 else nc.scalar
            eng.dma_start(w_sb[:, k], w_view[k])
        if xp_mode == "dve":
            idx_sb = resident.tile([BL, steps], I32)
            nc.sync.dma_start(idx_sb[:], idx_dram[:])
        else:
            xc_sb = resident.tile([120, 4 * H], BF16)
            nc.sync.dma_start(xc_sb[:], xc_dram[:])
            oh_sb = resident.tile([120, BL * steps], BF16)
            nc.sync.dma_start(oh_sb[:], oh_dram[:])

        # persistent per-quarter PSUM tiles: step t+1's quarter-q matmuls wait
        # only on step t's quarter-q gates-add (deterministic pipelining;
        # the pool's LIFO slot reuse would chain q0 onto q3's late free)
        ps_q = [psum_pool.tile([BL, 1024], F32, name=f"psq{i}") for i in range(NQ)]

        # rotating state buffers (3-deep c so the cell-store DMA never stalls
        # the c-update two steps later)
        c_st = [resident.tile([BL, H], F32, name=f"c{i}") for i in range(3)]
        hT_st = [resident.tile([128, NK, BL], BF16, name=f"hT{i}") for i in range(2)]

        xp_tiles = {}

        def issue_xp_gather(t):
            if xp_mode != "dve" or t >= steps:
                return
            xp = xp_pool.tile([BL, 4 * H], BF16, tag="xp", name=f"xp{t}")
            nc.gpsimd.indirect_dma_start(
                out=xp[:], out_offset=None, in_=xc_dram[:],
                in_offset=bass.IndirectOffsetOnAxis(ap=idx_sb[:, t:t + 1], axis=0),
            )
            xp_tiles[t] = xp

        for t in range(min(3, steps)):
            issue_xp_gather(t)

        for t in range(steps):
            c_old = c_st[(t + 2) % 3]   # written at t-1
            c_new = c_st[t % 3]
            hT_cur = hT_st[t % 2]
            hT_nxt = hT_st[(t + 1) % 2]

            hbf = h_pool.tile([BL, H], BF16, tag="hbf", name=f"hbf{t}")

            if xp_mode == "dve":
                xp = xp_tiles.pop(t)
                issue_xp_gather(t + 3)

            for q in range(NQ):
                qc = 1024 * q
                qs = slice(QH * q, QH * (q + 1))

                if xp_mode == "pe":
                    oh_t = oh_sb[:, BL * t:BL * (t + 1)]
                    ps = ps_q[q]
                    nc.tensor.matmul(ps[:, 0:512], oh_t, xc_sb[:, qc:qc + 512],
                                     start=True, stop=(t == 0))
                    nc.tensor.matmul(ps[:, 512:1024], oh_t,
                                     xc_sb[:, qc + 512:qc + 1024],
                                     start=True, stop=(t == 0))
                    if t > 0:
                        for k in range(NK):
                            nc.tensor.matmul(ps[:, 0:512], hT_cur[:, k],
                                             w_sb[:, k, qc:qc + 512],
                                             start=False, stop=(k == NK - 1))
                            nc.tensor.matmul(ps[:, 512:1024], hT_cur[:, k],
                                             w_sb[:, k, qc + 512:qc + 1024],
                                             start=False, stop=(k == NK - 1))
                    gate_src = ps
                else:
                    if t > 0:
                        ps = ps_q[q]
                        for k in range(NK):
                            nc.tensor.matmul(ps[:, 0:512], hT_cur[:, k],
                                             w_sb[:, k, qc:qc + 512],
                                             start=(k == 0), stop=(k == NK - 1))
                            nc.tensor.matmul(ps[:, 512:1024], hT_cur[:, k],
                                             w_sb[:, k, qc + 512:qc + 1024],
                                             start=(k == 0), stop=(k == NK - 1))
                        gate_src = act_pool.tile([BL, 1024], F32, tag="gates",
                                                 name=f"gates{t}_{q}")
                        # split adds per PSUM bank so [i|f] activations start
                        # one bank earlier than [g|o]
                        nc.vector.tensor_add(gate_src[:, 0:512], ps[:, 0:512],
                                             xp[:, qc:qc + 512])
                        nc.vector.tensor_add(gate_src[:, 512:1024], ps[:, 512:1024],
                                             xp[:, qc + 512:qc + 1024])
                    else:
                        gate_src = xp[:, qc:qc + 1024]

                # quarter layout: [i(256) | f(256) | g(256) | o(256)]
                if_t = act_pool.tile([BL, 2 * QH], F32, tag="if", name=f"if{t}_{q}")
                g_t = act_pool.tile([BL, QH], F32, tag="g", name=f"g{t}_{q}")
                nc.scalar.activation(g_t[:], gate_src[:, 512:768], Tanh)
                nc.scalar.activation(if_t[:, 0:256], gate_src[:, 0:256], Sigmoid)
                nc.scalar.activation(if_t[:, 256:512], gate_src[:, 256:512], Sigmoid)
                o_t = act_pool.tile([BL, QH], F32, tag="o", name=f"o{t}_{q}")
                nc.scalar.activation(o_t[:], gate_src[:, 768:1024], Sigmoid)

                if t == 0:
                    nc.vector.tensor_mul(c_new[:, qs], if_t[:, 0:256], g_t[:])
                else:
                    t1 = act_pool.tile([BL, QH], F32, tag="t1", name=f"t1{t}_{q}")
                    nc.vector.tensor_mul(t1[:], if_t[:, 0:256], g_t[:])
                    nc.vector.tensor_mul(c_new[:, qs], if_t[:, 256:512], c_old[:, qs])
                    nc.vector.tensor_add(c_new[:, qs], c_new[:, qs], t1[:])

                th = act_pool.tile([BL, QH], F32, tag="th", name=f"th{t}_{q}")
                nc.scalar.activation(th[:, 0:128], c_new[:, QH * q:QH * q + 128], Tanh)
                nc.scalar.activation(th[:, 128:256],
                                     c_new[:, QH * q + 128:QH * q + 256], Tanh)
                # h computed per 128-block so each transpose-DMA fires asap
                for half in range(2):
                    hs0 = QH * q + 128 * half
                    nc.vector.tensor_mul(hbf[:, hs0:hs0 + 128],
                                         o_t[:, 128 * half:128 * half + 128],
                                         th[:, 128 * half:128 * half + 128])
                    if t < steps - 1:
                        nc.sync.dma_start(hT_nxt[:, 2 * q + half],
                                          hbf[:, hs0:hs0 + 128], transpose=True)

            nc.gpsimd.dma_start(hid_dram[:, t, :], hbf[:])      # bf16 -> f32 cast DMA
            nc.scalar.dma_start(cell_dram[:, t, :], c_new[:])   # f32, HWDGE (ACT ring)

    nc.compile()
    return nc


def _host_prep(letter_seq, state_seq, letter_emb, state_emb, W_ih, W_hh, b_ih, b_hh,
               steps: int, xp_mode: str = XP_MODE):
    letter_seq = np.asarray(letter_seq)
    state_seq = np.asarray(state_seq)
    letter_emb = np.asarray(letter_emb, dtype=np.float32)
    state_emb = np.asarray(state_emb, dtype=np.float32)
    W_ih = np.asarray(W_ih, dtype=np.float32)
    W_hh = np.asarray(W_hh, dtype=np.float32)
    b_ih = np.asarray(b_ih, dtype=np.float32)
    b_hh = np.asarray(b_hh, dtype=np.float32)

    # column permutation: new col j = q*1024 + blk*256 + r  ->  orig 4H row
    q_idx = np.arange(4 * H) // 1024
    blk = (np.arange(4 * H) % 1024) // QH
    r = np.arange(4 * H) % QH
    colmap = np.array(GATE_BLOCKS)[blk] * H + q_idx * QH + r  # [4H]

    Wp = np.ascontiguousarray(W_hh[colmap, :].T).astype(ml_dtypes.bfloat16)  # [H, 4H]

    XL = letter_emb @ W_ih[:, :E].T                            # [30, 4H]
    XS = state_emb @ W_ih[:, E:].T                             # [4, 4H]
    bias = b_ih + b_hh
    XC = (XL[:, None, :] + XS[None, :, :] + bias).reshape(120, 4 * H)
    XC = np.ascontiguousarray(XC[:, colmap]).astype(ml_dtypes.bfloat16)  # [120, 4H]

    idx = (letter_seq.astype(np.int64) * 4 + state_seq.astype(np.int64))  # [B, S]
    in_maps = []
    for c in range(NCORES):
        idx_c = idx[BL * c:BL * (c + 1), :steps]               # [BL, steps]
        m = {"W": Wp, "XC": XC}
        if xp_mode == "dve":
            m["IDX"] = np.ascontiguousarray(idx_c).astype(np.int32)
        else:
            oh = np.zeros((120, BL * steps), dtype=ml_dtypes.bfloat16)
            cols = np.arange(BL * steps)
            oh[idx_c.T.reshape(-1), cols] = 1.0                # col = t*BL + b
            m["OH"] = oh
        in_maps.append(m)
    return in_maps


def kernel(letter_seq, state_seq, letter_emb, state_emb, W_ih, W_hh, b_ih, b_hh,
           steps: int = S):
    key = (steps, XP_MODE)
    if key not in _cache:
        _cache[key] = _build(steps, XP_MODE)
    nc = _cache[key]

    in_maps = _host_prep(letter_seq, state_seq, letter_emb, state_emb,
                         W_ih, W_hh, b_ih, b_hh, steps, XP_MODE)
    res = run_bass_kernel_spmd(nc, in_maps, core_ids=list(range(NCORES)))

    hidden = np.concatenate([res.results[c]["hid"] for c in range(NCORES)], axis=0)
    cell = np.concatenate([res.results[c]["cell"] for c in range(NCORES)], axis=0)
    return hidden, cell



# revision 22
# speedup vs baseline: 2.0874x; 2.0874x over previous
"""LSTM encoder kernel for Trainium2 (8 NeuronCores, data-parallel over batch).

Layout trick: SBUF/PSUM partition p = 32*q + b (q = H-quarter, b = batch row),
so the 32-batch-per-core problem fills all 128 partitions. The recurrent
matmul h @ W_hh.T runs as 4 CONCURRENT col-strip matmuls (PE tile_position
col-tiling, one 32-wide strip per quarter, each streaming its own W slice
over a separate XBUS) — ~4x the PE throughput of a single M=32 matmul chain.
The input projection is injected into the same PSUM accumulation via a
one-hot matmul against a 120-row combined-embedding table XC (30 letter
tokens x 4 state tokens), so no per-step gather DMA and no DVE add.

Per step:   psum[32q+b, n] = OH_t.T @ XC_q  +  sum_k hT[k].T @ W[k]_q
            (4 strips x 2 banks; per-quarter gate col layout [g|i|f|o])
            ACT: tanh(g), sigm(i,f,o), tanh(c');  DVE: c' = f*c + i*g, h = o*th
            h fed back via 8 blocked bf16 transpose-DMAs into hT[128, k, 32].
"""

from contextlib import ExitStack

import ml_dtypes
import numpy as np

import concourse.bacc as bacc
import concourse.mybir as mybir
import concourse.tile as tile
from concourse.bass_utils import run_bass_kernel_spmd

F32 = mybir.dt.float32
BF16 = mybir.dt.bfloat16

B, S, E, H = 256, 256, 256, 1024
NCORES = 8
BL = B // NCORES           # 32 batch rows per core
NK = H // 128              # 8 contraction tiles
NQ = 4                     # H quarters == PE col-strips
QH = H // NQ               # 256 h-units per quarter
GATE_OF_BLK = [2, 0, 1, 3]  # per-quarter col blocks [g|i|f|o] -> pytorch i,f,g,o rows
KORDER = [0, 2, 4, 6, 1, 3, 5, 7]  # consume hT blocks in production order (halves)

_cache = {}


def _build(steps: int, repeat: int = 1, bench: bool = False,
           diag_no_feedback: bool = False, diag_no_stores: bool = False,
           diag_no_elementwise: bool = False):
    """Emit the kernel. bench=True wraps the step loop in a hardware For_i
    repeat loop and stores outputs to a small circular buffer (same per-step
    device work, tiny host I/O) for differential wall-clock timing."""
    nc = bacc.Bacc("TRN2", target_bir_lowering=False, debug=False,
                   enable_asserts=not bench)

    w_dram = nc.dram_tensor("W", [H, 4 * H], BF16, kind="ExternalInput")
    xc_dram = nc.dram_tensor("XC", [120, 4 * H], BF16, kind="ExternalInput")
    oh_dram = nc.dram_tensor("OH", [120, BL * steps], BF16, kind="ExternalInput")
    eye_dram = nc.dram_tensor("EYE", [128, 128], BF16, kind="ExternalInput")
    out_steps = 8 if bench else steps
    # outputs in device layout [p=32q+b, t, u]; host reassembles to [BL, t, H]
    hid_dram = nc.dram_tensor("hid", [NQ * BL, out_steps, QH], F32,
                              kind="ExternalOutput")
    cell_dram = nc.dram_tensor("cell", [NQ * BL, out_steps, QH], F32,
                               kind="ExternalOutput")

    Tanh = mybir.ActivationFunctionType.Tanh
    Sigmoid = mybir.ActivationFunctionType.Sigmoid

    hid_view = hid_dram.ap()
    cell_view = cell_dram.ap()

    with tile.TileContext(nc) as tc, ExitStack() as ctx:
        resident = ctx.enter_context(tc.tile_pool(name="resident", bufs=1))
        psum_pool = ctx.enter_context(tc.tile_pool(name="psum", bufs=1, space="PSUM"))

        w_sb = resident.tile([128, NK, 4 * H], BF16)
        w_view = w_dram.ap().rearrange("(k p) n -> k p n", p=128)
        for k in range(NK):
            eng = nc.sync if k % 2 == 0 else nc.scalar
            eng.dma_start(w_sb[:, k], w_view[k])
        xc_sb = resident.tile([120, 4 * H], BF16)
        nc.sync.dma_start(xc_sb[:], xc_dram[:])
        oh_sb = resident.tile([120, BL * steps], BF16)
        nc.scalar.dma_start(oh_sb[:], oh_dram[:])
        eye_sb = resident.tile([128, 128], BF16)
        nc.sync.dma_start(eye_sb[:], eye_dram[:])

        # PSUM: 2 banks (g|i / f|o) x 2 step parities + 2 transpose staging
        ps_st = [[psum_pool.tile([128, 512], F32, name=f"ps{p}{b}") for b in range(2)]
                 for p in range(2)]
        tr_ps = [psum_pool.tile([128, 128], BF16, name=f"tr{h}") for h in range(2)]
        # rotating state (explicit rotation; all periods divide 256)
        c_st = [resident.tile([128, QH], F32, name=f"c{i}") for i in range(4)]
        # hT free layout [half, q, b]: one [128,128] transpose-DMA per half
        # lands all four strips' k-blocks (k = 2q + half) at once
        hT_st = [resident.tile([128, 2, NQ, BL], BF16, name=f"hT{i}") for i in range(2)]
        g_st = [resident.tile([128, QH], F32, name=f"g{i}") for i in range(2)]
        i_st = [resident.tile([128, QH], F32, name=f"i{i}") for i in range(2)]
        f_st = [resident.tile([128, QH], F32, name=f"f{i}") for i in range(2)]
        o_st = [resident.tile([128, QH], F32, name=f"o{i}") for i in range(2)]
        t1_st = [resident.tile([128, QH], F32, name=f"t1{i}") for i in range(2)]
        th_st = [resident.tile([128, QH], F32, name=f"th{i}") for i in range(2)]
        hbf_st = [resident.tile([128, QH], BF16, name=f"hbf{i}") for i in range(4)]

        if diag_no_elementwise:
            diag_no_feedback = True
        if diag_no_feedback:
            # hT never written by the loop; seed once so tiles aren't
            # read-before-write (timing diagnostic only, numerics invalid)
            for st in hT_st:
                for half in range(2):
                    nc.vector.tensor_copy(st[:, half], w_sb[:, 0, 0:128])

        # deferred PE-transpose of half 1: emitted mid-way through the NEXT
        # step's MM stream (so the in-order PE queue doesn't stall on it)
        pending_tr1 = []

        def emit_transpose(t, half):
            hbf = hbf_st[t % 4]
            hT_nxt = hT_st[(t + 1) % 2]
            hs = 128 * half
            nc.tensor.transpose(tr_ps[half][:], hbf[:, hs:hs + 128], eye_sb[:])
            nc.vector.tensor_copy(hT_nxt[:, half], tr_ps[half][:])

        def emit_xp(t):
            # one-hot(idx_t) x XC -> PSUM (opens the accumulation group);
            # hoisted one step early so it fills the PE during step t-1's tail
            par = t % 2
            pa, pb = ps_st[par]
            oh_t = oh_sb[:, BL * t:BL * (t + 1)]
            for beta, pt in ((0, pa), (1, pb)):
                for q in range(NQ):
                    nc.tensor.matmul(
                        pt[32 * q:32 * q + 32, :], oh_t,
                        xc_sb[:, 1024 * q + 512 * beta:1024 * q + 512 * beta + 512],
                        start=True, stop=(t == 0), tile_position=(0, 32 * q))

        def step_body(t):
            par = t % 2
            pa, pb = ps_st[par]
            hT_cur = hT_st[t % 2]

            def mm(pt, beta, k, q):
                nc.tensor.matmul(
                    pt[32 * q:32 * q + 32, :], hT_cur[:, k % 2, k // 2],
                    w_sb[:, k, 1024 * q + 512 * beta:1024 * q + 512 * beta + 512],
                    start=False, stop=(k == KORDER[-1]),
                    tile_position=(0, 32 * q))

            if t == 0:
                emit_xp(0)
            else:
                for beta, pt in ((0, pa), (1, pb)):
                    for k in (0, 2, 4, 6):          # evens: need only hT half 0
                        for q in range(NQ):
                            mm(pt, beta, k, q)
                while pending_tr1:
                    pending_tr1.pop()()              # prev step's half-1 transpose
                for beta, pt in ((0, pa), (1, pb)):
                    for k in (1, 3, 5, 7):
                        for q in range(NQ):
                            mm(pt, beta, k, q)
            if t + 1 < steps:
                emit_xp(t + 1)
            if diag_no_elementwise:
                return

            # elementwise, full 128-partition width
            g_t, i_t = g_st[par], i_st[par]
            f_t, o_t = f_st[par], o_st[par]
            t1 = t1_st[par]
            c_new, c_old = c_st[t % 4], c_st[(t + 3) % 4]
            hbf = hbf_st[t % 4]
            th = th_st[par]

            nc.scalar.activation(g_t[:], pa[:, 0:QH], Tanh)
            nc.scalar.activation(i_t[:], pa[:, QH:2 * QH], Sigmoid)
            if t == 0:
                nc.vector.tensor_mul(c_new[:], i_t[:], g_t[:])
                nc.scalar.activation(o_t[:], pb[:, QH:2 * QH], Sigmoid)
            else:
                nc.vector.tensor_mul(t1[:], i_t[:], g_t[:])
                nc.scalar.activation(f_t[:], pb[:, 0:QH], Sigmoid)
                nc.scalar.activation(o_t[:], pb[:, QH:2 * QH], Sigmoid)
                for hs in (0, 128):  # half-split c chain: th half0 starts earlier
                    nc.vector.tensor_mul(c_new[:, hs:hs + 128], f_t[:, hs:hs + 128],
                                         c_old[:, hs:hs + 128])
                    nc.vector.tensor_add(c_new[:, hs:hs + 128], c_new[:, hs:hs + 128],
                                         t1[:, hs:hs + 128])

            feedback = (t < steps - 1 or bench) and not diag_no_feedback
            for half in range(2):
                hs = 128 * half
                nc.scalar.activation(th[:, hs:hs + 128], c_new[:, hs:hs + 128], Tanh)
                nc.vector.tensor_mul(hbf[:, hs:hs + 128], o_t[:, hs:hs + 128],
                                     th[:, hs:hs + 128])
                if feedback:
                    if half == 0:
                        emit_transpose(t, 0)
                    else:
                        pending_tr1.append(lambda tt=t: emit_transpose(tt, 1))

            if not diag_no_stores:
                ot = t % out_steps
                nc.gpsimd.dma_start(hid_view[:, ot], hbf[:])  # bf16 -> f32 cast DMA
                nc.scalar.dma_start(cell_view[:, ot], c_new[:])

        if bench and repeat > 1:
            with tc.For_i(0, repeat):
                for t in range(steps):
                    step_body(t)
                while pending_tr1:   # close the feedback before loop back-edge
                    pending_tr1.pop()()
        else:
            for t in range(steps):
                step_body(t)
            pending_tr1.clear()

    nc.compile()
    return nc


def _host_prep(letter_seq, state_seq, letter_emb, state_emb, W_ih, W_hh, b_ih, b_hh,
               steps: int):
    letter_seq = np.asarray(letter_seq)
    state_seq = np.asarray(state_seq)
    letter_emb = np.asarray(letter_emb, dtype=np.float32)
    state_emb = np.asarray(state_emb, dtype=np.float32)
    W_ih = np.asarray(W_ih, dtype=np.float32)
    W_hh = np.asarray(W_hh, dtype=np.float32)
    b_ih = np.asarray(b_ih, dtype=np.float32)
    b_hh = np.asarray(b_hh, dtype=np.float32)

    # new col n = q*1024 + blk*256 + u  ->  orig 4H row (gate-major, i,f,g,o)
    n = np.arange(4 * H)
    q_idx, blk, u = n // 1024, (n % 1024) // QH, n % QH
    colmap = np.array(GATE_OF_BLK)[blk] * H + q_idx * QH + u  # [4H]

    Wp = np.ascontiguousarray(W_hh[colmap, :].T).astype(ml_dtypes.bfloat16)  # [H, 4H]

    XL = letter_emb @ W_ih[:, :E].T                            # [30, 4H]
    XS = state_emb @ W_ih[:, E:].T                             # [4, 4H]
    bias = b_ih + b_hh
    XC = (XL[:, None, :] + XS[None, :, :] + bias).reshape(120, 4 * H)
    XC = np.ascontiguousarray(XC[:, colmap]).astype(ml_dtypes.bfloat16)  # [120, 4H]

    idx = (letter_seq.astype(np.int64) * 4 + state_seq.astype(np.int64))  # [B, S]
    eye = np.eye(128, dtype=ml_dtypes.bfloat16)
    in_maps = []
    for c in range(NCORES):
        idx_c = idx[BL * c:BL * (c + 1), :steps]               # [BL, steps]
        oh = np.zeros((120, BL * steps), dtype=ml_dtypes.bfloat16)
        cols = np.arange(BL * steps)
        oh[idx_c.T.reshape(-1), cols] = 1.0                    # col = t*BL + b
        in_maps.append({"W": Wp, "XC": XC, "OH": oh, "EYE": eye})
    return in_maps


def kernel(letter_seq, state_seq, letter_emb, state_emb, W_ih, W_hh, b_ih, b_hh,
           steps: int = S):
    if steps not in _cache:
        _cache[steps] = _build(steps)
    nc = _cache[steps]

    in_maps = _host_prep(letter_seq, state_seq, letter_emb, state_emb,
                         W_ih, W_hh, b_ih, b_hh, steps)
    res = run_bass_kernel_spmd(nc, in_maps, core_ids=list(range(NCORES)))

    def unshuffle(a):
        # [4q*32b, steps, 256u] -> [32b, steps, 1024h]
        return np.ascontiguousarray(
            a.reshape(NQ, BL, steps, QH).transpose(1, 2, 0, 3).reshape(BL, steps, H)
        )

    hidden = np.concatenate(
        [unshuffle(res.results[c]["hid"]) for c in range(NCORES)], axis=0)
    cell = np.concatenate(
        [unshuffle(res.results[c]["cell"]) for c in range(NCORES)], axis=0)
    return hidden, cell


# revision 24
# speedup vs baseline: 2.1264x; 1.0187x over previous
"""LSTM encoder kernel for Trainium2 (8 NeuronCores, data-parallel over batch).

Layout trick: SBUF/PSUM partition p = 32*q + b (q = H-quarter, b = batch row),
so the 32-batch-per-core problem fills all 128 partitions. The recurrent
matmul h @ W_hh.T runs as 4 CONCURRENT col-strip matmuls (PE tile_position
col-tiling, one 32-wide strip per quarter, each streaming its own W slice
over a separate XBUS) — ~4x the PE throughput of a single M=32 matmul chain.
The input projection is injected into the same PSUM accumulation via a
one-hot matmul against a 120-row combined-embedding table XC (30 letter
tokens x 4 state tokens), so no per-step gather DMA and no DVE add.

Per step:   psum[32q+b, n] = OH_t.T @ XC_q  +  sum_k hT[k].T @ W[k]_q
            (4 strips x 2 banks; per-quarter gate col layout [g|i|f|o])
            ACT: tanh(g), sigm(i,f,o), tanh(c');  DVE: c' = f*c + i*g, h = o*th
The h feedback transposes on the PE itself (tensor.transpose vs identity,
~0.3us while the PE is idle waiting for hT anyway; a transpose-DMA costs
~4-5us of latency here) + a DVE copy PSUM->SBUF. One [128,128] transpose
per h-half lands all four strips' k-blocks at once (k = 2q + half); the
half-1 transpose is emitted mid-way through the NEXT step's MM stream so
the in-order PE queue doesn't stall on it, and next-step MMs consume hT
halves in production order (k evens, then odds). The next step's one-hot
xp matmuls are hoisted before the transposes in the PE queue to fill the
tail. Measured ~6.7us/step on TRN2 (PE span ~3.7us + exposed ACT/DVE/PE
feedback chain ~3.0us).
"""

from contextlib import ExitStack

import ml_dtypes
import numpy as np

import concourse.bacc as bacc
import concourse.mybir as mybir
import concourse.tile as tile
from concourse.bass_utils import run_bass_kernel_spmd

F32 = mybir.dt.float32
BF16 = mybir.dt.bfloat16

B, S, E, H = 256, 256, 256, 1024
NCORES = 8
BL = B // NCORES           # 32 batch rows per core
NK = H // 128              # 8 contraction tiles
NQ = 4                     # H quarters == PE col-strips
QH = H // NQ               # 256 h-units per quarter
GATE_OF_BLK = [2, 0, 1, 3]  # per-quarter col blocks [g|i|f|o] -> pytorch i,f,g,o rows
KORDER = [0, 2, 4, 6, 1, 3, 5, 7]  # consume hT blocks in production order (halves)

_cache = {}


def _build(steps: int, repeat: int = 1, bench: bool = False,
           diag_no_feedback: bool = False, diag_no_stores: bool = False,
           diag_no_elementwise: bool = False):
    """Emit the kernel. bench=True wraps the step loop in a hardware For_i
    repeat loop and stores outputs to a small circular buffer (same per-step
    device work, tiny host I/O) for differential wall-clock timing."""
    nc = bacc.Bacc("TRN2", target_bir_lowering=False, debug=False,
                   enable_asserts=not bench)

    w_dram = nc.dram_tensor("W", [H, 4 * H], BF16, kind="ExternalInput")
    xc_dram = nc.dram_tensor("XC", [120, 4 * H], BF16, kind="ExternalInput")
    oh_dram = nc.dram_tensor("OH", [120, BL * steps], BF16, kind="ExternalInput")
    eye_dram = nc.dram_tensor("EYE", [128, 128], BF16, kind="ExternalInput")
    out_steps = 8 if bench else steps
    # outputs in device layout [p=32q+b, t, u]; host reassembles to [BL, t, H]
    hid_dram = nc.dram_tensor("hid", [NQ * BL, out_steps, QH], F32,
                              kind="ExternalOutput")
    cell_dram = nc.dram_tensor("cell", [NQ * BL, out_steps, QH], F32,
                               kind="ExternalOutput")

    Tanh = mybir.ActivationFunctionType.Tanh
    Sigmoid = mybir.ActivationFunctionType.Sigmoid

    hid_view = hid_dram.ap()
    cell_view = cell_dram.ap()

    with tile.TileContext(nc) as tc, ExitStack() as ctx:
        resident = ctx.enter_context(tc.tile_pool(name="resident", bufs=1))
        psum_pool = ctx.enter_context(tc.tile_pool(name="psum", bufs=1, space="PSUM"))

        w_sb = resident.tile([128, NK, 4 * H], BF16)
        w_view = w_dram.ap().rearrange("(k p) n -> k p n", p=128)
        for k in range(NK):
            eng = nc.sync if k % 2 == 0 else nc.scalar
            eng.dma_start(w_sb[:, k], w_view[k])
        xc_sb = resident.tile([120, 4 * H], BF16)
        nc.sync.dma_start(xc_sb[:], xc_dram[:])
        oh_sb = resident.tile([120, BL * steps], BF16)
        nc.scalar.dma_start(oh_sb[:], oh_dram[:])
        eye_sb = resident.tile([128, 128], BF16)
        nc.sync.dma_start(eye_sb[:], eye_dram[:])

        # PSUM: 2 banks (g|i / f|o) x 2 step parities + 2 transpose staging
        ps_st = [[psum_pool.tile([128, 512], F32, name=f"ps{p}{b}") for b in range(2)]
                 for p in range(2)]
        tr_ps = [psum_pool.tile([128, 128], BF16, name=f"tr{h}") for h in range(2)]
        # rotating state (explicit rotation; all periods divide 256)
        c_st = [resident.tile([128, QH], F32, name=f"c{i}") for i in range(4)]
        # hT free layout [half, q, b]: one [128,128] transpose-DMA per half
        # lands all four strips' k-blocks (k = 2q + half) at once
        hT_st = [resident.tile([128, 2, NQ, BL], BF16, name=f"hT{i}") for i in range(2)]
        g_st = [resident.tile([128, QH], F32, name=f"g{i}") for i in range(2)]
        i_st = [resident.tile([128, QH], F32, name=f"i{i}") for i in range(2)]
        f_st = [resident.tile([128, QH], F32, name=f"f{i}") for i in range(2)]
        o_st = [resident.tile([128, QH], F32, name=f"o{i}") for i in range(2)]
        t1_st = [resident.tile([128, QH], F32, name=f"t1{i}") for i in range(2)]
        th_st = [resident.tile([128, QH], F32, name=f"th{i}") for i in range(2)]
        hbf_st = [resident.tile([128, QH], BF16, name=f"hbf{i}") for i in range(4)]

        if diag_no_elementwise:
            diag_no_feedback = True
        if diag_no_feedback:
            # hT never written by the loop; seed once so tiles aren't
            # read-before-write (timing diagnostic only, numerics invalid)
            for st in hT_st:
                for half in range(2):
                    nc.vector.tensor_copy(st[:, half], w_sb[:, 0, 0:128])

        # deferred PE-transpose of half 1: emitted mid-way through the NEXT
        # step's MM stream (so the in-order PE queue doesn't stall on it)
        pending_tr1 = []

        def emit_transpose(t, half):
            hbf = hbf_st[t % 4]
            hT_nxt = hT_st[(t + 1) % 2]
            hs = 128 * half
            nc.tensor.transpose(tr_ps[half][:], hbf[:, hs:hs + 128], eye_sb[:])
            nc.vector.tensor_copy(hT_nxt[:, half], tr_ps[half][:])

        def emit_xp(t):
            # one-hot(idx_t) x XC -> PSUM (opens the accumulation group);
            # hoisted one step early so it fills the PE during step t-1's tail
            par = t % 2
            pa, pb = ps_st[par]
            oh_t = oh_sb[:, BL * t:BL * (t + 1)]
            for beta, pt in ((0, pa), (1, pb)):
                for q in range(NQ):
                    nc.tensor.matmul(
                        pt[32 * q:32 * q + 32, :], oh_t,
                        xc_sb[:, 1024 * q + 512 * beta:1024 * q + 512 * beta + 512],
                        start=True, stop=(t == 0), tile_position=(0, 32 * q))

        def step_body(t):
            par = t % 2
            pa, pb = ps_st[par]
            hT_cur = hT_st[t % 2]

            def mm(pt, beta, k, q):
                nc.tensor.matmul(
                    pt[32 * q:32 * q + 32, :], hT_cur[:, k % 2, k // 2],
                    w_sb[:, k, 1024 * q + 512 * beta:1024 * q + 512 * beta + 512],
                    start=False, stop=(k == KORDER[-1]),
                    tile_position=(0, 32 * q))

            if t == 0:
                emit_xp(0)
            else:
                for beta, pt in ((0, pa), (1, pb)):
                    for k in (0, 2, 4, 6):          # evens: need only hT half 0
                        for q in range(NQ):
                            mm(pt, beta, k, q)
                while pending_tr1:
                    pending_tr1.pop()()              # prev step's half-1 transpose
                for beta, pt in ((0, pa), (1, pb)):
                    for k in (1, 3, 5, 7):
                        for q in range(NQ):
                            mm(pt, beta, k, q)
            if t + 1 < steps:
                emit_xp(t + 1)
            if diag_no_elementwise:
                return

            # elementwise, full 128-partition width
            g_t, i_t = g_st[par], i_st[par]
            f_t, o_t = f_st[par], o_st[par]
            t1 = t1_st[par]
            c_new, c_old = c_st[t % 4], c_st[(t + 3) % 4]
            hbf = hbf_st[t % 4]
            th = th_st[par]

            feedback = (t < steps - 1 or bench) and not diag_no_feedback

            def c_half(hs):
                nc.vector.tensor_mul(c_new[:, hs:hs + 128], f_t[:, hs:hs + 128],
                                     c_old[:, hs:hs + 128])
                nc.vector.tensor_add(c_new[:, hs:hs + 128], c_new[:, hs:hs + 128],
                                     t1[:, hs:hs + 128])

            def h_half(half):
                hs = 128 * half
                nc.scalar.activation(th[:, hs:hs + 128], c_new[:, hs:hs + 128], Tanh)
                nc.vector.tensor_mul(hbf[:, hs:hs + 128], o_t[:, hs:hs + 128],
                                     th[:, hs:hs + 128])
                if feedback:
                    if half == 0:
                        emit_transpose(t, 0)
                    else:
                        pending_tr1.append(lambda tt=t: emit_transpose(tt, 1))

            nc.scalar.activation(g_t[:], pa[:, 0:QH], Tanh)
            nc.scalar.activation(i_t[:], pa[:, QH:2 * QH], Sigmoid)
            if t == 0:
                nc.vector.tensor_mul(c_new[:], i_t[:], g_t[:])
                nc.scalar.activation(o_t[:], pb[:, QH:2 * QH], Sigmoid)
                for half in range(2):
                    h_half(half)
            else:
                nc.vector.tensor_mul(t1[:], i_t[:], g_t[:])
                nc.scalar.activation(f_t[:], pb[:, 0:QH], Sigmoid)
                nc.scalar.activation(o_t[:], pb[:, QH:2 * QH], Sigmoid)
                # critical half-0 chain first; half-1 c ops run after h0 is
                # off to the PE (they'd otherwise delay h0 in the DVE FIFO)
                c_half(0)
                h_half(0)
                c_half(128)
                h_half(1)

            if not diag_no_stores:
                ot = t % out_steps
                nc.gpsimd.dma_start(hid_view[:, ot], hbf[:])  # bf16 -> f32 cast DMA
                nc.scalar.dma_start(cell_view[:, ot], c_new[:])

        if bench and repeat > 1:
            with tc.For_i(0, repeat):
                for t in range(steps):
                    step_body(t)
                while pending_tr1:   # close the feedback before loop back-edge
                    pending_tr1.pop()()
        else:
            for t in range(steps):
                step_body(t)
            pending_tr1.clear()

    nc.compile()
    return nc


def _host_prep(letter_seq, state_seq, letter_emb, state_emb, W_ih, W_hh, b_ih, b_hh,
               steps: int):
    letter_seq = np.asarray(letter_seq)
    state_seq = np.asarray(state_seq)
    letter_emb = np.asarray(letter_emb, dtype=np.float32)
    state_emb = np.asarray(state_emb, dtype=np.float32)
    W_ih = np.asarray(W_ih, dtype=np.float32)
    W_hh = np.asarray(W_hh, dtype=np.float32)
    b_ih = np.asarray(b_ih, dtype=np.float32)
    b_hh = np.asarray(b_hh, dtype=np.float32)

    # new col n = q*1024 + blk*256 + u  ->  orig 4H row (gate-major, i,f,g,o)
    n = np.arange(4 * H)
    q_idx, blk, u = n // 1024, (n % 1024) // QH, n % QH
    colmap = np.array(GATE_OF_BLK)[blk] * H + q_idx * QH + u  # [4H]

    Wp = np.ascontiguousarray(W_hh[colmap, :].T).astype(ml_dtypes.bfloat16)  # [H, 4H]

    XL = letter_emb @ W_ih[:, :E].T                            # [30, 4H]
    XS = state_emb @ W_ih[:, E:].T                             # [4, 4H]
    bias = b_ih + b_hh
    XC = (XL[:, None, :] + XS[None, :, :] + bias).reshape(120, 4 * H)
    XC = np.ascontiguousarray(XC[:, colmap]).astype(ml_dtypes.bfloat16)  # [120, 4H]

    idx = (letter_seq.astype(np.int64) * 4 + state_seq.astype(np.int64))  # [B, S]
    eye = np.eye(128, dtype=ml_dtypes.bfloat16)
    in_maps = []
    for c in range(NCORES):
        idx_c = idx[BL * c:BL * (c + 1), :steps]               # [BL, steps]
        oh = np.zeros((120, BL * steps), dtype=ml_dtypes.bfloat16)
        cols = np.arange(BL * steps)
        oh[idx_c.T.reshape(-1), cols] = 1.0                    # col = t*BL + b
        in_maps.append({"W": Wp, "XC": XC, "OH": oh, "EYE": eye})
    return in_maps


def kernel(letter_seq, state_seq, letter_emb, state_emb, W_ih, W_hh, b_ih, b_hh,
           steps: int = S):
    if steps not in _cache:
        _cache[steps] = _build(steps)
    nc = _cache[steps]

    in_maps = _host_prep(letter_seq, state_seq, letter_emb, state_emb,
                         W_ih, W_hh, b_ih, b_hh, steps)
    res = run_bass_kernel_spmd(nc, in_maps, core_ids=list(range(NCORES)))

    def unshuffle(a):
        # [4q*32b, steps, 256u] -> [32b, steps, 1024h]
        return np.ascontiguousarray(
            a.reshape(NQ, BL, steps, QH).transpose(1, 2, 0, 3).reshape(BL, steps, H)
        )

    hidden = np.concatenate(
        [unshuffle(res.results[c]["hid"]) for c in range(NCORES)], axis=0)
    cell = np.concatenate(
        [unshuffle(res.results[c]["cell"]) for c in range(NCORES)], axis=0)
    return hidden, cell


# revision 29
# speedup vs baseline: 2.2699x; 1.0675x over previous
"""LSTM encoder kernel for Trainium2 (8 NeuronCores, data-parallel over batch).

Layout trick: SBUF/PSUM partition p = 32*q + b (q = H-quarter, b = batch row),
so the 32-batch-per-core problem fills all 128 partitions. The recurrent
matmul h @ W_hh.T runs as 4 CONCURRENT col-strip matmuls (PE tile_position
col-tiling, one 32-wide strip per quarter, each streaming its own W slice
over a separate XBUS) — ~4x the PE throughput of a single M=32 matmul chain.
The input projection is injected into the same PSUM accumulation via a
one-hot matmul against a 120-row combined-embedding table XC (30 letter
tokens x 4 state tokens), so no per-step gather DMA and no DVE add.

Per step:   psum[32q+b, n] = OH_t.T @ XC_q  +  sum_k hT[k].T @ W[k]_q
            (4 strips x 2 banks; per-quarter gate col layout [g|i|f|o])
            ACT: tanh(g), sigm(i,f,o), tanh(c');  DVE: c' = f*c + i*g, h = o*th
The h feedback transposes on the PE itself (tensor.transpose vs identity,
~0.3us while the PE is idle waiting for hT anyway; a transpose-DMA costs
~4-5us of latency here) + a DVE copy PSUM->SBUF. One [128,128] transpose
per h-half lands all four strips' k-blocks at once (k = 2q + half); the
half-1 transpose is emitted mid-way through the NEXT step's MM stream so
the in-order PE queue doesn't stall on it, and next-step MMs consume hT
halves in production order (k evens, then odds). The next step's one-hot
xp matmuls are hoisted before the transposes in the PE queue to fill the
tail. Measured ~6.7us/step on TRN2 (PE span ~3.7us + exposed ACT/DVE/PE
feedback chain ~3.0us).
"""

from contextlib import ExitStack

import ml_dtypes
import numpy as np

import concourse.bacc as bacc
import concourse.mybir as mybir
import concourse.tile as tile
from concourse.bass_utils import run_bass_kernel_spmd

F32 = mybir.dt.float32
BF16 = mybir.dt.bfloat16

B, S, E, H = 256, 256, 256, 1024
NCORES = 8
BL = B // NCORES           # 32 batch rows per core
NK = H // 128              # 8 contraction tiles
NQ = 4                     # H quarters == PE col-strips
QH = H // NQ               # 256 h-units per quarter
GATE_OF_BLK = [2, 0, 1, 3]  # per-quarter col blocks [g|i|f|o] -> pytorch i,f,g,o rows
KORDER = [0, 2, 4, 6, 1, 3, 5, 7]  # consume hT blocks in production order (halves)

_cache = {}


def _build(steps: int, repeat: int = 1, bench: bool = False,
           diag_no_feedback: bool = False, diag_no_stores: bool = False,
           diag_no_elementwise: bool = False):
    """Emit the kernel. bench=True wraps the step loop in a hardware For_i
    repeat loop and stores outputs to a small circular buffer (same per-step
    device work, tiny host I/O) for differential wall-clock timing."""
    nc = bacc.Bacc("TRN2", target_bir_lowering=False, debug=False,
                   enable_asserts=not bench)

    w_dram = nc.dram_tensor("W", [H, 4 * H], BF16, kind="ExternalInput")
    xc_dram = nc.dram_tensor("XC", [120, 4 * H], BF16, kind="ExternalInput")
    oh_dram = nc.dram_tensor("OH", [120, BL * steps], BF16, kind="ExternalInput")
    eye_dram = nc.dram_tensor("EYE", [128, 128], BF16, kind="ExternalInput")
    out_steps = 8 if bench else steps
    # outputs in device layout [p=32q+b, t, u]; host reassembles to [BL, t, H]
    hid_dram = nc.dram_tensor("hid", [NQ * BL, out_steps, QH], F32,
                              kind="ExternalOutput")
    cell_dram = nc.dram_tensor("cell", [NQ * BL, out_steps, QH], F32,
                               kind="ExternalOutput")

    Tanh = mybir.ActivationFunctionType.Tanh
    Sigmoid = mybir.ActivationFunctionType.Sigmoid

    hid_view = hid_dram.ap()
    cell_view = cell_dram.ap()

    with tile.TileContext(nc) as tc, ExitStack() as ctx:
        resident = ctx.enter_context(tc.tile_pool(name="resident", bufs=1))
        psum_pool = ctx.enter_context(tc.tile_pool(name="psum", bufs=1, space="PSUM"))

        w_sb = resident.tile([128, NK, 4 * H], BF16)
        w_view = w_dram.ap().rearrange("(k p) n -> k p n", p=128)
        for k in range(NK):
            eng = nc.sync if k % 2 == 0 else nc.scalar
            eng.dma_start(w_sb[:, k], w_view[k])
        xc_sb = resident.tile([120, 4 * H], BF16)
        nc.sync.dma_start(xc_sb[:], xc_dram[:])
        oh_sb = resident.tile([120, BL * steps], BF16)
        nc.scalar.dma_start(oh_sb[:], oh_dram[:])
        eye_sb = resident.tile([128, 128], BF16)
        nc.sync.dma_start(eye_sb[:], eye_dram[:])

        # PSUM: 2 banks (g|i / f|o) x 2 step parities + 2 transpose staging
        ps_st = [[psum_pool.tile([128, 512], F32, name=f"ps{p}{b}") for b in range(2)]
                 for p in range(2)]
        tr_ps = [psum_pool.tile([128, 128], BF16, name=f"tr{h}") for h in range(2)]
        # rotating state (explicit rotation; all periods divide 256)
        c_st = [resident.tile([128, QH], F32, name=f"c{i}") for i in range(4)]
        # hT free layout [half, q, b]: one [128,128] transpose-DMA per half
        # lands all four strips' k-blocks (k = 2q + half) at once
        hT_st = [resident.tile([128, 2, NQ, BL], BF16, name=f"hT{i}") for i in range(2)]
        g_st = [resident.tile([128, QH], F32, name=f"g{i}") for i in range(2)]
        i_st = [resident.tile([128, QH], F32, name=f"i{i}") for i in range(2)]
        f_st = [resident.tile([128, QH], F32, name=f"f{i}") for i in range(2)]
        o_st = [resident.tile([128, QH], F32, name=f"o{i}") for i in range(2)]
        t1_st = [resident.tile([128, QH], F32, name=f"t1{i}") for i in range(2)]
        th_st = [resident.tile([128, QH], F32, name=f"th{i}") for i in range(2)]
        hbf_st = [resident.tile([128, QH], BF16, name=f"hbf{i}") for i in range(4)]

        if diag_no_elementwise:
            diag_no_feedback = True
        if diag_no_feedback:
            # hT never written by the loop; seed once so tiles aren't
            # read-before-write (timing diagnostic only, numerics invalid)
            for st in hT_st:
                for half in range(2):
                    nc.vector.tensor_copy(st[:, half], w_sb[:, 0, 0:128])

        # deferred PE-transpose of half 1: emitted mid-way through the NEXT
        # step's MM stream (so the in-order PE queue doesn't stall on it)
        pending_tr1 = []

        def emit_transpose(t, half):
            hbf = hbf_st[t % 4]
            hT_nxt = hT_st[(t + 1) % 2]
            hs = 128 * half
            nc.tensor.transpose(tr_ps[half][:], hbf[:, hs:hs + 128], eye_sb[:])
            nc.vector.tensor_copy(hT_nxt[:, half], tr_ps[half][:])

        def emit_xp(t):
            # one-hot(idx_t) x XC -> PSUM (opens the accumulation group);
            # hoisted one step early so it fills the PE during step t-1's tail
            par = t % 2
            pa, pb = ps_st[par]
            oh_t = oh_sb[:, BL * t:BL * (t + 1)]
            for beta, pt in ((0, pa), (1, pb)):
                for q in range(NQ):
                    nc.tensor.matmul(
                        pt[32 * q:32 * q + 32, :], oh_t,
                        xc_sb[:, 1024 * q + 512 * beta:1024 * q + 512 * beta + 512],
                        start=True, stop=(t == 0), tile_position=(0, 32 * q))

        def step_body(t):
            par = t % 2
            pa, pb = ps_st[par]
            hT_cur = hT_st[t % 2]

            def mm(pt, beta, k, q):
                nc.tensor.matmul(
                    pt[32 * q:32 * q + 32, :], hT_cur[:, k % 2, k // 2],
                    w_sb[:, k, 1024 * q + 512 * beta:1024 * q + 512 * beta + 512],
                    start=False, stop=(k == KORDER[-1]),
                    tile_position=(0, 32 * q))

            if t == 0:
                emit_xp(0)
            else:
                for beta, pt in ((0, pa), (1, pb)):
                    for k in (0, 2, 4, 6):          # evens: need only hT half 0
                        for q in range(NQ):
                            mm(pt, beta, k, q)
                while pending_tr1:
                    pending_tr1.pop()()              # prev step's half-1 transpose
                for beta, pt in ((0, pa), (1, pb)):
                    for k in (1, 3, 5, 7):
                        for q in range(NQ):
                            mm(pt, beta, k, q)
            if t + 1 < steps:
                emit_xp(t + 1)
            if diag_no_elementwise:
                return

            # elementwise, full 128-partition width
            g_t, i_t = g_st[par], i_st[par]
            f_t, o_t = f_st[par], o_st[par]
            t1 = t1_st[par]
            c_new, c_old = c_st[t % 4], c_st[(t + 3) % 4]
            hbf = hbf_st[t % 4]
            th = th_st[par]

            feedback = (t < steps - 1 or bench) and not diag_no_feedback

            def c_half(hs):
                nc.vector.tensor_mul(c_new[:, hs:hs + 128], f_t[:, hs:hs + 128],
                                     c_old[:, hs:hs + 128])
                nc.vector.tensor_add(c_new[:, hs:hs + 128], c_new[:, hs:hs + 128],
                                     t1[:, hs:hs + 128])

            def h_half(half):
                hs = 128 * half
                nc.scalar.activation(th[:, hs:hs + 128], c_new[:, hs:hs + 128], Tanh)
                nc.vector.tensor_mul(hbf[:, hs:hs + 128], o_t[:, hs:hs + 128],
                                     th[:, hs:hs + 128])
                if feedback:
                    if half == 0:
                        emit_transpose(t, 0)
                    else:
                        pending_tr1.append(lambda tt=t: emit_transpose(tt, 1))

            nc.scalar.activation(g_t[:], pa[:, 0:QH], Tanh)
            nc.scalar.activation(i_t[:], pa[:, QH:2 * QH], Sigmoid)
            if t == 0:
                nc.vector.tensor_mul(c_new[:], i_t[:], g_t[:])
                nc.scalar.activation(o_t[:], pb[:, QH:2 * QH], Sigmoid)
                for half in range(2):
                    h_half(half)
            else:
                nc.vector.tensor_mul(t1[:], i_t[:], g_t[:])
                nc.scalar.activation(f_t[:], pb[:, 0:QH], Sigmoid)
                nc.scalar.activation(o_t[:], pb[:, QH:2 * QH], Sigmoid)
                # critical half-0 chain first; half-1 c ops run after h0 is
                # off to the PE (they'd otherwise delay h0 in the DVE FIFO)
                c_half(0)
                h_half(0)
                c_half(128)
                h_half(1)

            if not diag_no_stores:
                ot = t % out_steps
                nc.gpsimd.dma_start(hid_view[:, ot], hbf[:])  # bf16 -> f32 cast DMA
                nc.scalar.dma_start(cell_view[:, ot], c_new[:])

        if bench and repeat > 1:
            with tc.For_i(0, repeat):
                for t in range(steps):
                    step_body(t)
                while pending_tr1:   # close the feedback before loop back-edge
                    pending_tr1.pop()()
        else:
            for t in range(steps):
                step_body(t)
            pending_tr1.clear()

    nc.compile()
    return nc


def _host_prep(letter_seq, state_seq, letter_emb, state_emb, W_ih, W_hh, b_ih, b_hh,
               steps: int):
    letter_seq = np.asarray(letter_seq)
    state_seq = np.asarray(state_seq)
    letter_emb = np.asarray(letter_emb, dtype=np.float32)
    state_emb = np.asarray(state_emb, dtype=np.float32)
    W_ih = np.asarray(W_ih, dtype=np.float32)
    W_hh = np.asarray(W_hh, dtype=np.float32)
    b_ih = np.asarray(b_ih, dtype=np.float32)
    b_hh = np.asarray(b_hh, dtype=np.float32)

    # new col n = q*1024 + blk*256 + u  ->  orig 4H row (gate-major, i,f,g,o)
    n = np.arange(4 * H)
    q_idx, blk, u = n // 1024, (n % 1024) // QH, n % QH
    colmap = np.array(GATE_OF_BLK)[blk] * H + q_idx * QH + u  # [4H]

    Wp = np.ascontiguousarray(W_hh[colmap, :].T).astype(ml_dtypes.bfloat16)  # [H, 4H]

    XL = letter_emb @ W_ih[:, :E].T                            # [30, 4H]
    XS = state_emb @ W_ih[:, E:].T                             # [4, 4H]
    bias = b_ih + b_hh
    XC = (XL[:, None, :] + XS[None, :, :] + bias).reshape(120, 4 * H)
    XC = np.ascontiguousarray(XC[:, colmap]).astype(ml_dtypes.bfloat16)  # [120, 4H]

    idx = (letter_seq.astype(np.int64) * 4 + state_seq.astype(np.int64))  # [B, S]
    eye = np.eye(128, dtype=ml_dtypes.bfloat16)
    in_maps = []
    for c in range(NCORES):
        idx_c = idx[BL * c:BL * (c + 1), :steps]               # [BL, steps]
        oh = np.zeros((120, BL * steps), dtype=ml_dtypes.bfloat16)
        cols = np.arange(BL * steps)
        oh[idx_c.T.reshape(-1), cols] = 1.0                    # col = t*BL + b
        in_maps.append({"W": Wp, "XC": XC, "OH": oh, "EYE": eye})
    return in_maps


def kernel(letter_seq, state_seq, letter_emb, state_emb, W_ih, W_hh, b_ih, b_hh,
           steps: int = S):
    if steps not in _cache:
        _cache[steps] = _build(steps)
    nc = _cache[steps]

    in_maps = _host_prep(letter_seq, state_seq, letter_emb, state_emb,
                         W_ih, W_hh, b_ih, b_hh, steps)
    res = run_bass_kernel_spmd(nc, in_maps, core_ids=list(range(NCORES)))

    def unshuffle(a):
        # [4q*32b, steps, 256u] -> [32b, steps, 1024h]
        return np.ascontiguousarray(
            a.reshape(NQ, BL, steps, QH).transpose(1, 2, 0, 3).reshape(BL, steps, H)
        )

    hidden = np.concatenate(
        [unshuffle(res.results[c]["hid"]) for c in range(NCORES)], axis=0)
    cell = np.concatenate(
        [unshuffle(res.results[c]["cell"]) for c in range(NCORES)], axis=0)
    return hidden, cell
